# revision 16
# baseline (speedup 1.0000x reference)
"""Trainium2 Bass kernel for nn_CANet: 30-iteration neural cellular automaton.

cell [32,16,128,128] f32; per iteration:
    pre  = maxpool3x3(cell[:, :1]) > 0.1
    perc = [cell, dwconv(cell, sobel_x), dwconv(cell, sobel_y)]   # 48 ch
    h    = relu(w1 @ perc); cur = w2 @ h + cell
    post = maxpool3x3(cur[:, :1]) > 0.1
    cell = cur * (pre & post)
Output cell[:, 1:11] after 30 iterations.  Data parallel: 4 images/core x 8.

A layout: CELL [128p, 66*130]: p = 32q + 16g + ch, q = 2*pair + hh.
Image j = 2*pair + g; free = 66 rows x 130 cols (zero gutters + seam halos).
Ping-pong state (CELL_A/CELL_B).

5-tap perception: since sobel_x = vsmooth (x) hdiff and sobel_y = vdiff (x)
hsmooth, the folded 3x3 tap weights satisfy F(0,dx)=F(-1,dx)+F(1,dx) and
F(dy,0)=F(dy,-1)+F(dy,1).  With R = 2x2 block-sum of cell (R[r,u] =
sum cell[r..r+1, u..u+1], built via bf16 P = horizontal pair-sum):
    w1 @ perc = W1a@cell + sum_{ty,tx in 0,1} F(2ty-1,2tx-1) @ R(r-1+ty,u-1+tx)
i.e. 4 bf16 corner taps on R + 1 f32r center tap on cell (vs 9 full taps).

Per iteration (src S -> dst D): taps (5) accumulate h into PSUM per
(quarter, chunk) with 4-way PE row tiling; relu evicts to bf16 H tiles
(engines rotated ACT/DVE/Pool); W2 bf16 col-tiled matmuls -> cur PSUM;
evict cur + S -> D (DVE/Pool).  Mask: spread cur ch0 to B layout, maxpool
+ threshold x PREB fused (DVE), mask back to bf16 A-plane, ones-matmul
broadcast + multiply apply (DVE/Pool), then P/R plane rebuild per chunk.
"""
import os
import sys

sys.path.insert(0, "/opt/trn_rl_repo")

import numpy as np

H = W = 128
C_IN = 10
CH = 16
NIMG = 4
NCORES = int(os.environ.get("CANET_CORES", "8"))
NITER = int(os.environ.get("CANET_NITER", "30"))
UNROLL = int(os.environ.get("CANET_UNROLL", "2"))
ROWS = 66
COLS = 130
FREE = ROWS * COLS
NCH = 16                  # chunks (4-row blocks) per half-image
CR = 4                    # rows per chunk
N = CR * W                # 512
GC = 2                    # chunks per taps group
LIVE_THR = 0.1

_BUILT = None

# engine rotation patterns (tunable). Pool ('p') cannot access PSUM, so
# relu/evict/apply (PSUM-sourced) may only use 's' (ACT) / 'v' (DVE).
RELU_PAT = os.environ.get("CANET_RELU_PAT", "sssssv")
EVICT_PAT = os.environ.get("CANET_EVICT_PAT", "v")
APPLY_PAT = os.environ.get("CANET_APPLY_PAT", "v")
P_PAT = os.environ.get("CANET_P_PAT", "p")
R_PAT = os.environ.get("CANET_R_PAT", "v")
WARM = int(os.environ.get("CANET_WARM", "16"))


def _sobel():
    s = np.array([[-1.0, 0.0, 1.0], [-2.0, 0.0, 2.0], [-1.0, 0.0, 1.0]],
                 dtype=np.float32) / 8.0
    return s, np.ascontiguousarray(s.T)


def _host_weights(w1, w2):
    import ml_dtypes
    sx, sy = _sobel()
    # center tap: W1a (f32), block diag per (q, g)
    wtsc = np.zeros((128, 96), np.float32)
    # 4 corner taps on R (bf16): t = 2*ty + tx, (dy,dx) = (2ty-1, 2tx-1)
    wtsr = np.zeros((128, 4 * 96), np.float32)
    for t in range(4):
        dy, dx = 2 * (t // 2) - 1, 2 * (t % 2) - 1
        F = w1[:, 16:32] * sx[dy + 1, dx + 1] + w1[:, 32:48] * sy[dy + 1, dx + 1]
        for q in range(4):
            for g in range(2):
                p0 = 32 * q + 16 * g
                wtsr[p0:p0 + 16, t * 96 + 48 * g:t * 96 + 48 * g + 48] = F.T
                if t == 0:
                    wtsc[p0:p0 + 16, 48 * g:48 * g + 48] = w1[:, 0:16].T
    # W2 col-block slices, bf16
    wts2 = np.zeros((128, 128), np.float32)
    for g in range(2):
        for qp in range(4):
            wts2[48 * g:48 * g + 48,
                 32 * qp + 16 * g:32 * qp + 16 * g + 16] = w2.T
    ones = np.zeros((128, 128), np.float32)
    for q in range(4):
        for g in range(2):
            p = 32 * q + 16 * g
            ones[p, p:p + 16] = 1.0
    bf = ml_dtypes.bfloat16
    return (wtsc, wtsr.astype(bf), wts2.astype(bf), ones.astype(bf))


def _host_cell0(inp4):
    cell = np.zeros((NIMG, CH, H, W), np.float32)
    cell[:, 1:C_IN + 1] = inp4
    cell[:, 0] = 1.0 - inp4[:, 0]
    dev = np.zeros((128, FREE), np.float32)
    for pair in range(2):
        for hh in range(2):
            q = 2 * pair + hh
            for g in range(2):
                j = 2 * pair + g
                for r in range(ROWS):
                    y = 64 * hh + r - 1
                    if 0 <= y < H:
                        dev[32 * q + 16 * g:32 * q + 16 * g + CH,
                            r * COLS + 1:r * COLS + 1 + W] = cell[j, :, y, :]
    return dev, cell


def _host_preb0(cell):
    import ml_dtypes
    x = cell[:, 0]
    p = np.full((NIMG, H + 2, W + 2), -np.inf, np.float32)
    p[:, 1:-1, 1:-1] = x
    mp = None
    for dy in range(3):
        for dx in range(3):
            win = p[:, dy:dy + H, dx:dx + W]
            mp = win.copy() if mp is None else np.maximum(mp, win)
    pre = (mp > LIVE_THR).astype(np.float32)
    out = np.zeros((128, 512), np.float32)
    for j in range(NIMG):
        for hb in range(32):
            out[32 * j + hb] = pre[j, 4 * hb:4 * hb + 4, :].reshape(-1)
    return out.astype(ml_dtypes.bfloat16)


def _split_waits(nc):
    import concourse.mybir as mybir
    nsplit = 0
    for fn in nc.m.functions:
        for bb in fn.blocks:
            new_list = []
            for ins in bb.instructions:
                si = ins.sync_info
                if si is not None and si.on_wait and len(si.on_wait) > 1:
                    waits = list(si.on_wait)
                    for w2 in waits[:-1]:
                        nop = mybir.InstNoOp(name=f"{ins.name}-ws{nsplit}")
                        nop.engine = ins.engine
                        nop.sync_info = mybir.SyncInfo(on_wait=[w2], on_update=[])
                        new_list.append(nop)
                        nsplit += 1
                    ins.sync_info = mybir.SyncInfo(on_wait=[waits[-1]],
                                                  on_update=list(si.on_update))
                new_list.append(ins)
            bb.instructions = new_list
    return nsplit


def _build():
    import concourse.bass as bass
    import concourse.mybir as mybir
    import concourse.tile as tile

    f32 = mybir.dt.float32
    bf16 = mybir.dt.bfloat16
    f32r = mybir.dt.float32r

    nc = bass.Bass()
    cell0_in = nc.dram_tensor("cell0", [128, FREE], f32, kind="ExternalInput")
    wtsc_in = nc.dram_tensor("wtsc", [128, 96], f32, kind="ExternalInput")
    wtsr_in = nc.dram_tensor("wtsr", [128, 384], bf16, kind="ExternalInput")
    wts2_in = nc.dram_tensor("wts2", [128, 128], bf16, kind="ExternalInput")
    ones_in = nc.dram_tensor("ones", [128, 128], bf16, kind="ExternalInput")
    preb0_in = nc.dram_tensor("preb0", [128, 512], bf16, kind="ExternalInput")
    out_ext = nc.dram_tensor("out", [NIMG, C_IN, H, W], f32,
                             kind="ExternalOutput")

    AT = mybir.AluOpType

    with tile.TileContext(nc) as tc:
        with (
            tc.tile_pool(name="state", bufs=1) as sp,
            tc.tile_pool(name="hbuf", bufs=10) as hbp,
            tc.tile_pool(name="psum", bufs=8, space="PSUM") as pp,
        ):
            CELLA = sp.tile([128, FREE], f32r)
            CELLB = sp.tile([128, FREE], f32r)
            PPL = sp.tile([128, FREE], bf16)        # horizontal pair-sums
            RPLA = sp.tile([128, FREE], bf16)       # 2x2 block sums of CELLA
            RPLB = sp.tile([128, FREE], bf16)
            WTSC = sp.tile([128, 96], f32r)
            WTSR = sp.tile([128, 384], bf16)
            WTS2 = sp.tile([128, 128], bf16)
            ONES = sp.tile([128, 128], bf16)
            MPLANE = sp.tile([128, FREE], bf16)
            B = sp.tile([128, 6 * COLS], f32)
            B2 = sp.tile([128, 6 * COLS], f32)
            PREB = sp.tile([128, 512], bf16)
            VM = sp.tile([128, 4 * COLS], f32)
            HM = sp.tile([128, 512], f32)
            MB = sp.tile([128, 512], bf16)

            with tc.tile_pool(name="init", bufs=1) as ip:
                C0 = ip.tile([128, FREE], f32)
                W0 = ip.tile([128, 96], f32)
                nc.sync.dma_start(C0[:], cell0_in[:])
                nc.vector.tensor_copy(CELLA[:], C0[:])
                nc.sync.dma_start(W0[:], wtsc_in[:])
                nc.vector.tensor_copy(WTSC[:], W0[:])
            nc.sync.dma_start(WTSR[:], wtsr_in[:])
            nc.sync.dma_start(WTS2[:], wts2_in[:])
            nc.sync.dma_start(ONES[:], ones_in[:])
            nc.sync.dma_start(PREB[:], preb0_in[:])
            nc.gpsimd.memset(MPLANE[:], 0.0)
            nc.gpsimd.memset(B2[:], 0.0)
            nc.gpsimd.memset(CELLB[:].bitcast(f32), 0.0)
            nc.gpsimd.memset(PPL[:], 0.0)
            nc.gpsimd.memset(RPLA[:], 0.0)
            nc.gpsimd.memset(RPLB[:], 0.0)

            mg = MPLANE[:].rearrange("p (r c) -> p r c", c=COLS)
            bg = B[:].rearrange("p (r c) -> p r c", c=COLS)
            b2g = B2[:].rearrange("p (r c) -> p r c", c=COLS)
            vg = VM[:].rearrange("p (r c) -> p r c", c=COLS)
            hmg = HM[:].rearrange("p (r c) -> p r c", c=W)

            ENG = {"v": nc.vector, "p": nc.gpsimd}

            def relu_engine(k):
                c = RELU_PAT[k % len(RELU_PAT)]
                return c

            def do_relu(k, dst, src):
                c = relu_engine(k)
                if c == "s":
                    nc.scalar.activation(dst, src,
                                         mybir.ActivationFunctionType.Relu)
                else:
                    ENG[c].tensor_relu(dst, src)

            def p_chunk(S, P, i, eng):
                """P rows of chunk i (flat 2D APs so DVE pack modes engage):
                P[r,u] = S[r,u]+S[r,u+1]; col 129 = gutter+gutter = 0."""
                sf = S[:].bitcast(f32)
                a = (1 + CR * i) * COLS
                b = a + CR * COLS
                eng.tensor_tensor(P[:, a:b], sf[:, a:b], sf[:, a + 1:b + 1],
                                  op=AT.add)

            def p_seam(S, P, r, eng):
                sf = S[:].bitcast(f32)
                a = r * COLS
                w = COLS if r < ROWS - 1 else COLS - 1
                eng.tensor_tensor(P[:, a:a + w], sf[:, a:a + w],
                                  sf[:, a + 1:a + 1 + w], op=AT.add)

            def r_chunk(P, R, i, eng):
                """R rows {4i..4i+3(+1 for i=15)}: R[r,u] = P[r,u]+P[r+1,u]."""
                a = CR * i * COLS
                b = a + (5 if i == NCH - 1 else 4) * COLS
                eng.tensor_tensor(R[:, a:b], P[:, a:b],
                                  P[:, a + COLS:b + COLS], op=AT.add)

            def pr_planes(S, P, R):
                """Full P/R plane build (prologue only)."""
                for r in (0, ROWS - 1):
                    p_seam(S, P, r, nc.vector)
                for i in range(NCH):
                    p_chunk(S, P, i, ENG[P_PAT[i % len(P_PAT)]])
                for i in range(NCH):
                    r_chunk(P, R, i, ENG[R_PAT[i % len(R_PAT)]])

            def make_tail(D, RD):
                """Mask-phase closures for state D (bcast+apply+P+R per
                chunk).  step(c) for c in 0..15, then finale()."""
                dg = D[:].rearrange("p (r c) -> p r c", c=COLS)
                dgf = D[:].bitcast(f32).rearrange("p (r c) -> p r c", c=COLS)

                def step(c):
                    r0 = 1 + CR * c
                    mp = pp.tile([128, N], f32, tag="ps", name="mp")
                    nc.tensor.matmul(
                        mp[:, :], ONES[:, :],
                        mg[:, r0:r0 + CR, 1:1 + W],
                        start=True, stop=True)
                    ENG[APPLY_PAT[c % len(APPLY_PAT)]].tensor_tensor(
                        dg[:, r0:r0 + CR, 1:1 + W],
                        dgf[:, r0:r0 + CR, 1:1 + W],
                        mp[:, :], op=AT.mult)
                    p_chunk(D, PPL, c, ENG[P_PAT[c % len(P_PAT)]])
                    if c == 0:
                        # masked seam row 65 (reads D row 1 = chunk 0) ->
                        # P seam row 65, needed by R(15)
                        for pair in range(2):
                            q0, q1 = 2 * pair, 2 * pair + 1
                            nc.sync.dma_start(
                                dg[32 * q0:32 * q0 + 32, 65, :],
                                dg[32 * q1:32 * q1 + 32, 1, :])
                        p_seam(D, PPL, 65, nc.gpsimd)
                    if c >= 1:
                        r_chunk(PPL, RD, c, ENG[R_PAT[c % len(R_PAT)]])

                def finale():
                    # masked seam row 0 (reads D row 64 = chunk 15) ->
                    # P seam row 0 -> R(0); pass-A runs chunk 0 last
                    for pair in range(2):
                        q0, q1 = 2 * pair, 2 * pair + 1
                        nc.sync.dma_start(dg[32 * q1:32 * q1 + 32, 0, :],
                                          dg[32 * q0:32 * q0 + 32, 64, :])
                    p_seam(D, PPL, 0, nc.gpsimd)
                    r_chunk(PPL, RD, 0, ENG[R_PAT[0]])
                return step, finale

            def pass_a(S, D, RS, with_tail):
                """Taps/relu/W2/evict S->D, chunk order [1..15, 0].  If
                with_tail, interleave the mask phase of the PREVIOUS
                transition (which masks S and builds RS) two chunks ahead
                of the pass-A chunk that consumes it."""
                sg = S[:].rearrange("p (r c) -> p r c", c=COLS)
                sgf = S[:].bitcast(f32).rearrange("p (r c) -> p r c", c=COLS)
                dg = D[:].rearrange("p (r c) -> p r c", c=COLS)
                rsg = RS[:].rearrange("p (r c) -> p r c", c=COLS)
                tail = make_tail(S, RS) if with_tail else None

                # PE warm-keeper: dep-free matmuls bridging the prep-phase
                # PE idle gap so HAM doesn't re-throttle the clock
                if WARM:
                    jp = pp.tile([128, N], f32, tag="ps", name="jp")
                    for _ in range(WARM):
                        nc.tensor.matmul(jp[0:128, 0:384], ONES[0:32, :],
                                         WTSR[0:32, :],
                                         start=True, stop=True)
                if tail:
                    for c in range(3):
                        tail[0](c)

                relu_k = 0
                pend = None          # (i, H tiles) awaiting W2+evict

                def w2_evict(p):
                    i, Hs = p
                    r0 = 1 + CR * i
                    cp = pp.tile([128, N], f32, tag="ps", name="cp")
                    for q in range(4):
                        nc.tensor.matmul(
                            cp[32 * q:32 * q + 32, :],
                            WTS2[0:96, 32 * q:32 * q + 32],
                            Hs[q][0:96, :],
                            start=True, stop=True,
                            tile_position=(0, 32 * q))
                    ENG[EVICT_PAT[i % len(EVICT_PAT)]].scalar_tensor_tensor(
                        dg[:, r0:r0 + CR, 1:1 + W],
                        cp[:, :], 0.0,
                        sgf[:, r0:r0 + CR, 1:1 + W],
                        op0=AT.add, op1=AT.add)

                for idx, i in enumerate(list(range(1, NCH)) + [0]):
                    r0 = 1 + CR * i
                    hps = [pp.tile([128, N], f32, tag="ps", name="hp")
                           for q in range(4)]
                    # 5 taps, t-major, weight-stationary per (t,q)
                    for t in range(5):
                        for q in range(4):
                            if t == 0:
                                lhsT = WTSC[32 * q:32 * q + 32, 0:96]
                                rhs = sg[32 * q:32 * q + 32,
                                         r0:r0 + CR, 1:1 + W]
                            else:
                                ty, tx = (t - 1) // 2, (t - 1) % 2
                                lhsT = WTSR[32 * q:32 * q + 32,
                                            (t - 1) * 96:(t - 1) * 96 + 96]
                                rhs = rsg[32 * q:32 * q + 32,
                                          r0 - 1 + ty:r0 - 1 + ty + CR,
                                          tx:tx + W]
                            nc.tensor.matmul(
                                hps[q][0:96, :], lhsT, rhs,
                                start=(t == 0), stop=(t == 4),
                                tile_position=(32 * q, 0))
                    # W2+evict of the previous chunk (its relus are done by
                    # now, so the PE never waits on ACT/DVE)
                    if pend is not None:
                        w2_evict(pend)
                    Hs = []
                    for q in range(4):
                        Ht = hbp.tile([128, N], bf16, tag="H", name="Ht")
                        do_relu(relu_k, Ht[0:96, :], hps[q][0:96, :])
                        relu_k += 1
                        Hs.append(Ht)
                    pend = (i, Hs)
                    if tail:
                        nxt = idx + 3
                        if nxt < NCH:
                            tail[0](nxt)
                        elif nxt == NCH:
                            tail[1]()
                w2_evict(pend)

            def prep(D, first=False):
                """Mask prep for D: unmasked seams, ch0 spread, maxpool,
                MB = post&pre, mask-back DMAs.  Also emits the deferred
                next-pre (B2) path of the previous transition first."""
                dg = D[:].rearrange("p (r c) -> p r c", c=COLS)
                if not first:
                    # previous transition's next-pre maxpool (reads old B
                    # and MB, writes PREB consumed by this MB op)
                    nc.vector.tensor_tensor(
                        b2g[:, 1:5, 1:1 + W], bg[:, 1:5, 1:1 + W],
                        MB[:].rearrange("p (r c) -> p r c", c=W), op=AT.mult)
                    for j in range(NIMG):
                        nc.sync.dma_start(b2g[32 * j:32 * j + 31, 5, :],
                                          b2g[32 * j + 1:32 * j + 32, 1, :])
                        nc.sync.dma_start(b2g[32 * j + 1:32 * j + 32, 0, :],
                                          b2g[32 * j:32 * j + 31, 4, :])
                    nc.vector.tensor_tensor(VM[:, :], B2[:, 0:4 * COLS],
                                            B2[:, COLS:5 * COLS], op=AT.max)
                    nc.vector.tensor_tensor(VM[:, :], VM[:, :],
                                            B2[:, 2 * COLS:6 * COLS],
                                            op=AT.max)
                    nc.vector.tensor_tensor(hmg, vg[:, :, 0:W],
                                            vg[:, :, 1:1 + W], op=AT.max)
                    nc.vector.tensor_tensor(hmg, hmg, vg[:, :, 2:2 + W],
                                            op=AT.max)
                    nc.vector.tensor_single_scalar(PREB[:, :], HM[:, :],
                                                   LIVE_THR, op=AT.is_gt)
                # unmasked seams on D (for spread halos)
                for pair in range(2):
                    q0, q1 = 2 * pair, 2 * pair + 1
                    nc.sync.dma_start(dg[32 * q1:32 * q1 + 32, 0, :],
                                      dg[32 * q0:32 * q0 + 32, 64, :])
                    nc.sync.dma_start(dg[32 * q0:32 * q0 + 32, 65, :],
                                      dg[32 * q1:32 * q1 + 32, 1, :])
                # spread cur ch0 -> B
                engs = [nc.sync, nc.scalar]
                k = 0
                for pair in range(2):
                    for hh in range(2):
                        q = 2 * pair + hh
                        for g in range(2):
                            j = 2 * pair + g
                            p = 32 * q + 16 * g
                            srcb = D[p:p + 1].bitcast(f32)
                            src = bass.AP(
                                srcb.tensor, srcb.offset,
                                [list(srcb.ap[0]), [4 * COLS, 16], [COLS, 6],
                                 [1, COLS]])
                            engs[k % 2].dma_start(
                                B[32 * j + 16 * hh:32 * j + 16 * hh + 16, :]
                                .rearrange("p (r c) -> p r c", c=COLS), src)
                            k += 1
                # maxpool + threshold, fused with pre -> MB
                nc.vector.tensor_tensor(VM[:, :], B[:, 0:4 * COLS],
                                        B[:, COLS:5 * COLS], op=AT.max)
                nc.vector.tensor_tensor(VM[:, :], VM[:, :],
                                        B[:, 2 * COLS:6 * COLS], op=AT.max)
                nc.vector.tensor_tensor(hmg, vg[:, :, 0:W], vg[:, :, 1:1 + W],
                                        op=AT.max)
                nc.vector.tensor_tensor(hmg, hmg, vg[:, :, 2:2 + W], op=AT.max)
                nc.vector.scalar_tensor_tensor(MB[:, :], HM[:, :], LIVE_THR,
                                               PREB[:, :],
                                               op0=AT.is_gt, op1=AT.mult)
                # m back to bf16 A plane
                engs2 = [nc.sync, nc.scalar]
                k = 0
                for pair in range(2):
                    for hh in range(2):
                        q = 2 * pair + hh
                        for g in range(2):
                            j = 2 * pair + g
                            p = 32 * q + 16 * g
                            engs2[k % 2].dma_start(
                                mg[p:p + 1, 1:65, 1:1 + W],
                                MB[32 * j + 16 * hh:32 * j + 16 * hh + 16, :])
                            k += 1

            def plain_tail(S, RS):
                stepf, finalef = make_tail(S, RS)
                for c in range(NCH):
                    stepf(c)
                finalef()

            # prologue: P/R planes of the initial state + transition 1
            pr_planes(CELLA, PPL, RPLA)
            assert NITER >= 4 and NITER % 2 == 0
            nbody = (NITER - 2) // 2
            pass_a(CELLA, CELLB, RPLA, with_tail=False)
            prep(CELLB, first=True)
            if os.environ.get("CANET_LOOP", "fori") == "fori" and nbody > 1:
                with tc.For_i(0, nbody, 1, hint_engines=(
                        mybir.EngineType.PE, mybir.EngineType.DVE,
                        mybir.EngineType.Activation, mybir.EngineType.SP,
                        mybir.EngineType.Pool)):
                    pass_a(CELLB, CELLA, RPLB, with_tail=True)
                    prep(CELLA)
                    pass_a(CELLA, CELLB, RPLA, with_tail=True)
                    prep(CELLB)
            else:
                for _ in range(nbody):
                    pass_a(CELLB, CELLA, RPLB, with_tail=True)
                    prep(CELLA)
                    pass_a(CELLA, CELLB, RPLA, with_tail=True)
                    prep(CELLB)
            # epilogue: transition 30 + final mask
            pass_a(CELLB, CELLA, RPLB, with_tail=True)
            prep(CELLA)
            plain_tail(CELLA, RPLA)
            FINAL = CELLA

            fgf = FINAL[:].bitcast(f32).rearrange("p (r c) -> p r c", c=COLS)
            for pair in range(2):
                for hh in range(2):
                    q = 2 * pair + hh
                    for g in range(2):
                        j = 2 * pair + g
                        nc.sync.dma_start(
                            out_ext[j, :, 64 * hh:64 * hh + 64, :],
                            fgf[32 * q + 16 * g + 1:32 * q + 16 * g + 11,
                                1:65, 1:1 + W])

    nc.finalize()
    if os.environ.get("CANET_SPLIT", "1") == "1":
        _split_waits(nc)
    return nc


def _get_built():
    global _BUILT
    if _BUILT is None:
        _BUILT = _build()
    return _BUILT


def _host_inputs(inp, w1, w2):
    wtsc, wtsr, wts2, ones = _host_weights(w1, w2)
    in_maps = []
    for core in range(NCORES):
        inp4 = inp[NIMG * core:NIMG * core + NIMG]
        cell0_dev, cell = _host_cell0(inp4)
        preb0 = _host_preb0(cell)
        in_maps.append({"cell0": cell0_dev, "wtsc": wtsc, "wtsr": wtsr,
                        "wts2": wts2, "ones": ones, "preb0": preb0})
    return in_maps


def kernel(inp, w1, w2):
    inp = np.asarray(inp, np.float32)
    w1 = np.asarray(w1, np.float32)
    w2 = np.asarray(w2, np.float32)
    from concourse.bass_utils import run_bass_kernel_spmd

    nc = _get_built()
    in_maps = _host_inputs(inp, w1, w2)
    res = run_bass_kernel_spmd(nc, in_maps, list(range(NCORES))).results
    out = np.concatenate([res[c]["out"] for c in range(NCORES)], axis=0)
    return np.ascontiguousarray(out.astype(np.float32))


# revision 19
# speedup vs baseline: 1.2924x; 1.2924x over previous
"""Trainium2 Bass kernel for nn_CANet: 30-iteration neural cellular automaton.

cell [32,16,128,128] f32; per iteration:
    pre  = maxpool3x3(cell[:, :1]) > 0.1
    perc = [cell, dwconv(cell, sobel_x), dwconv(cell, sobel_y)]   # 48 ch
    h    = relu(w1 @ perc); cur = w2 @ h + cell
    post = maxpool3x3(cur[:, :1]) > 0.1
    cell = cur * (pre & post)
Output cell[:, 1:11] after 30 iterations.  Data parallel: 4 images/core x 8.

A layout: CELL [128p, 66*130]: p = 32q + 16g + ch, q = 2*pair + hh.
Image j = 2*pair + g; free = 66 rows x 130 cols (zero gutters + seam halos).
Ping-pong state (CELL_A/CELL_B).

5-tap perception: since sobel_x = vsmooth (x) hdiff and sobel_y = vdiff (x)
hsmooth, the folded 3x3 tap weights satisfy F(0,dx)=F(-1,dx)+F(1,dx) and
F(dy,0)=F(dy,-1)+F(dy,1).  With R = 2x2 block-sum of cell (R[r,u] =
sum cell[r..r+1, u..u+1], built via bf16 P = horizontal pair-sum):
    w1 @ perc = W1a@cell + sum_{ty,tx in 0,1} F(2ty-1,2tx-1) @ R(r-1+ty,u-1+tx)
i.e. 4 bf16 corner taps on R + 1 f32r center tap on cell (vs 9 full taps).

Per iteration (src S -> dst D): taps (5) accumulate h into PSUM per
(quarter, chunk) with 4-way PE row tiling; relu evicts to bf16 H tiles
(engines rotated ACT/DVE/Pool); W2 bf16 col-tiled matmuls -> cur PSUM;
evict cur + S -> D (DVE/Pool).  Mask: spread cur ch0 to B layout, maxpool
+ threshold x PREB fused (DVE), mask back to bf16 A-plane, ones-matmul
broadcast + multiply apply (DVE/Pool), then P/R plane rebuild per chunk.
"""
import os
import sys

sys.path.insert(0, "/opt/trn_rl_repo")

import numpy as np

H = W = 128
C_IN = 10
CH = 16
NIMG = 4
NCORES = int(os.environ.get("CANET_CORES", "8"))
NITER = int(os.environ.get("CANET_NITER", "30"))
UNROLL = int(os.environ.get("CANET_UNROLL", "2"))
ROWS = 66
COLS = 130
FREE = ROWS * COLS
NCH = 16                  # chunks (4-row blocks) per half-image
CR = 4                    # rows per chunk
N = CR * W                # 512
GC = 2                    # chunks per taps group
LIVE_THR = 0.1

_BUILT = None

# engine rotation patterns (tunable). Pool ('p') cannot access PSUM, so
# relu/evict/apply (PSUM-sourced) may only use 's' (ACT) / 'v' (DVE).
RELU_PAT = os.environ.get("CANET_RELU_PAT", "sssssv")
EVICT_PAT = os.environ.get("CANET_EVICT_PAT", "v")
APPLY_PAT = os.environ.get("CANET_APPLY_PAT", "v")
P_PAT = os.environ.get("CANET_P_PAT", "p")
R_PAT = os.environ.get("CANET_R_PAT", "v")
WARM = int(os.environ.get("CANET_WARM", "16"))
AHEAD = int(os.environ.get("CANET_AHEAD", "5"))


def _sobel():
    s = np.array([[-1.0, 0.0, 1.0], [-2.0, 0.0, 2.0], [-1.0, 0.0, 1.0]],
                 dtype=np.float32) / 8.0
    return s, np.ascontiguousarray(s.T)


def _host_weights(w1, w2):
    import ml_dtypes
    sx, sy = _sobel()
    # center tap: W1a (f32), block diag per (q, g)
    wtsc = np.zeros((128, 96), np.float32)
    # 4 corner taps on R (bf16): t = 2*ty + tx, (dy,dx) = (2ty-1, 2tx-1)
    wtsr = np.zeros((128, 4 * 96), np.float32)
    for t in range(4):
        dy, dx = 2 * (t // 2) - 1, 2 * (t % 2) - 1
        F = w1[:, 16:32] * sx[dy + 1, dx + 1] + w1[:, 32:48] * sy[dy + 1, dx + 1]
        for q in range(4):
            for g in range(2):
                p0 = 32 * q + 16 * g
                wtsr[p0:p0 + 16, t * 96 + 48 * g:t * 96 + 48 * g + 48] = F.T
                if t == 0:
                    wtsc[p0:p0 + 16, 48 * g:48 * g + 48] = w1[:, 0:16].T
    # W2 col-block slices, bf16
    wts2 = np.zeros((128, 128), np.float32)
    for g in range(2):
        for qp in range(4):
            wts2[48 * g:48 * g + 48,
                 32 * qp + 16 * g:32 * qp + 16 * g + 16] = w2.T
    # per-chunk broadcast matrices: out partition p (= 32q+16g+ch) reads
    # MB partition 32j+16hh+c  (j = 2*(q//2)+g, hh = q%2)
    onesb = np.zeros((128, NCH * 128), np.float32)
    for c in range(NCH):
        for q in range(4):
            for g in range(2):
                j = 2 * (q // 2) + g
                hh = q % 2
                kb = 32 * j + 16 * hh + c
                for ch in range(16):
                    onesb[kb, c * 128 + 32 * q + 16 * g + ch] = 1.0
    bf = ml_dtypes.bfloat16
    return (wtsc, wtsr.astype(bf), wts2.astype(bf), onesb.astype(bf))


def _host_cell0(inp4):
    cell = np.zeros((NIMG, CH, H, W), np.float32)
    cell[:, 1:C_IN + 1] = inp4
    cell[:, 0] = 1.0 - inp4[:, 0]
    dev = np.zeros((128, FREE), np.float32)
    for pair in range(2):
        for hh in range(2):
            q = 2 * pair + hh
            for g in range(2):
                j = 2 * pair + g
                for r in range(ROWS):
                    y = 64 * hh + r - 1
                    if 0 <= y < H:
                        dev[32 * q + 16 * g:32 * q + 16 * g + CH,
                            r * COLS + 1:r * COLS + 1 + W] = cell[j, :, y, :]
    return dev, cell


def _host_preb0(cell):
    import ml_dtypes
    x = cell[:, 0]
    p = np.full((NIMG, H + 2, W + 2), -np.inf, np.float32)
    p[:, 1:-1, 1:-1] = x
    mp = None
    for dy in range(3):
        for dx in range(3):
            win = p[:, dy:dy + H, dx:dx + W]
            mp = win.copy() if mp is None else np.maximum(mp, win)
    pre = (mp > LIVE_THR).astype(np.float32)
    out = np.zeros((128, 512), np.float32)
    for j in range(NIMG):
        for hb in range(32):
            out[32 * j + hb] = pre[j, 4 * hb:4 * hb + 4, :].reshape(-1)
    return out.astype(ml_dtypes.bfloat16)


def _split_waits(nc):
    import concourse.mybir as mybir
    nsplit = 0
    for fn in nc.m.functions:
        for bb in fn.blocks:
            new_list = []
            for ins in bb.instructions:
                si = ins.sync_info
                if si is not None and si.on_wait and len(si.on_wait) > 1:
                    waits = list(si.on_wait)
                    for w2 in waits[:-1]:
                        nop = mybir.InstNoOp(name=f"{ins.name}-ws{nsplit}")
                        nop.engine = ins.engine
                        nop.sync_info = mybir.SyncInfo(on_wait=[w2], on_update=[])
                        new_list.append(nop)
                        nsplit += 1
                    ins.sync_info = mybir.SyncInfo(on_wait=[waits[-1]],
                                                  on_update=list(si.on_update))
                new_list.append(ins)
            bb.instructions = new_list
    return nsplit


def _build():
    import concourse.bass as bass
    import concourse.mybir as mybir
    import concourse.tile as tile

    f32 = mybir.dt.float32
    bf16 = mybir.dt.bfloat16
    f32r = mybir.dt.float32r

    nc = bass.Bass()
    cell0_in = nc.dram_tensor("cell0", [128, FREE], f32, kind="ExternalInput")
    wtsc_in = nc.dram_tensor("wtsc", [128, 96], f32, kind="ExternalInput")
    wtsr_in = nc.dram_tensor("wtsr", [128, 384], bf16, kind="ExternalInput")
    wts2_in = nc.dram_tensor("wts2", [128, 128], bf16, kind="ExternalInput")
    onesb_in = nc.dram_tensor("onesb", [128, NCH * 128], bf16,
                              kind="ExternalInput")
    preb0_in = nc.dram_tensor("preb0", [128, 512], bf16, kind="ExternalInput")
    out_ext = nc.dram_tensor("out", [NIMG, C_IN, H, W], f32,
                             kind="ExternalOutput")

    AT = mybir.AluOpType

    with tile.TileContext(nc) as tc:
        with (
            tc.tile_pool(name="state", bufs=1) as sp,
            tc.tile_pool(name="hbuf", bufs=10) as hbp,
            tc.tile_pool(name="psum", bufs=8, space="PSUM") as pp,
        ):
            CELLA = sp.tile([128, FREE], f32r)
            CELLB = sp.tile([128, FREE], f32r)
            PPL = sp.tile([128, FREE], bf16)        # horizontal pair-sums
            RPLA = sp.tile([128, FREE], bf16)       # 2x2 block sums of CELLA
            RPLB = sp.tile([128, FREE], bf16)
            WTSC = sp.tile([128, 96], f32r)
            WTSR = sp.tile([128, 384], bf16)
            WTS2 = sp.tile([128, 128], bf16)
            ONESB = sp.tile([128, NCH * 128], bf16)
            B = sp.tile([128, 6 * COLS], f32)
            B2 = sp.tile([128, 6 * COLS], f32)
            PREB = sp.tile([128, 512], bf16)
            VM = sp.tile([128, 4 * COLS], f32)
            HM = sp.tile([128, 512], f32)
            MB = sp.tile([128, 512], bf16)

            with tc.tile_pool(name="init", bufs=1) as ip:
                C0 = ip.tile([128, FREE], f32)
                W0 = ip.tile([128, 96], f32)
                nc.sync.dma_start(C0[:], cell0_in[:])
                nc.vector.tensor_copy(CELLA[:], C0[:])
                nc.sync.dma_start(W0[:], wtsc_in[:])
                nc.vector.tensor_copy(WTSC[:], W0[:])
            nc.sync.dma_start(WTSR[:], wtsr_in[:])
            nc.sync.dma_start(WTS2[:], wts2_in[:])
            nc.sync.dma_start(ONESB[:], onesb_in[:])
            nc.sync.dma_start(PREB[:], preb0_in[:])
            nc.gpsimd.memset(B2[:], 0.0)
            nc.gpsimd.memset(CELLB[:].bitcast(f32), 0.0)
            nc.gpsimd.memset(PPL[:], 0.0)
            nc.gpsimd.memset(RPLA[:], 0.0)
            nc.gpsimd.memset(RPLB[:], 0.0)

            bg = B[:].rearrange("p (r c) -> p r c", c=COLS)
            b2g = B2[:].rearrange("p (r c) -> p r c", c=COLS)
            vg = VM[:].rearrange("p (r c) -> p r c", c=COLS)
            hmg = HM[:].rearrange("p (r c) -> p r c", c=W)

            ENG = {"v": nc.vector, "p": nc.gpsimd}

            def relu_engine(k):
                c = RELU_PAT[k % len(RELU_PAT)]
                return c

            def do_relu(k, dst, src):
                c = relu_engine(k)
                if c == "s":
                    nc.scalar.activation(dst, src,
                                         mybir.ActivationFunctionType.Relu)
                else:
                    ENG[c].tensor_relu(dst, src)

            def p_chunk(S, P, i, eng):
                """P rows of chunk i (flat 2D APs so DVE pack modes engage):
                P[r,u] = S[r,u]+S[r,u+1]; col 129 = gutter+gutter = 0."""
                sf = S[:].bitcast(f32)
                a = (1 + CR * i) * COLS
                b = a + CR * COLS
                eng.tensor_tensor(P[:, a:b], sf[:, a:b], sf[:, a + 1:b + 1],
                                  op=AT.add)

            def p_seam(S, P, r, eng):
                sf = S[:].bitcast(f32)
                a = r * COLS
                w = COLS if r < ROWS - 1 else COLS - 1
                eng.tensor_tensor(P[:, a:a + w], sf[:, a:a + w],
                                  sf[:, a + 1:a + 1 + w], op=AT.add)

            def r_chunk(P, R, i, eng):
                """R rows {4i..4i+3(+1 for i=15)}: R[r,u] = P[r,u]+P[r+1,u]."""
                a = CR * i * COLS
                b = a + (5 if i == NCH - 1 else 4) * COLS
                eng.tensor_tensor(R[:, a:b], P[:, a:b],
                                  P[:, a + COLS:b + COLS], op=AT.add)

            def pr_planes(S, P, R):
                """Full P/R plane build (prologue only)."""
                for r in (0, ROWS - 1):
                    p_seam(S, P, r, nc.vector)
                for i in range(NCH):
                    p_chunk(S, P, i, ENG[P_PAT[i % len(P_PAT)]])
                for i in range(NCH):
                    r_chunk(P, R, i, ENG[R_PAT[i % len(R_PAT)]])

            def make_tail(D, RD):
                """Mask-phase closures for state D (bcast+apply+P+R per
                chunk).  step(c) for c in 0..15, then finale()."""
                dg = D[:].rearrange("p (r c) -> p r c", c=COLS)
                dgf = D[:].bitcast(f32).rearrange("p (r c) -> p r c", c=COLS)

                def step(c):
                    r0 = 1 + CR * c
                    mp = pp.tile([128, N], f32, tag="ps", name="mp")
                    nc.tensor.matmul(
                        mp[:, :], ONESB[:, 128 * c:128 * c + 128],
                        MB[:, :],
                        start=True, stop=True)
                    ENG[APPLY_PAT[c % len(APPLY_PAT)]].tensor_tensor(
                        dg[:, r0:r0 + CR, 1:1 + W],
                        dgf[:, r0:r0 + CR, 1:1 + W],
                        mp[:, :], op=AT.mult)
                    p_chunk(D, PPL, c, ENG[P_PAT[c % len(P_PAT)]])
                    if c == 0:
                        # masked seam row 65 (reads D row 1 = chunk 0) ->
                        # P seam row 65, needed by R(15)
                        for pair in range(2):
                            q0, q1 = 2 * pair, 2 * pair + 1
                            nc.sync.dma_start(
                                dg[32 * q0:32 * q0 + 32, 65, :],
                                dg[32 * q1:32 * q1 + 32, 1, :])
                        p_seam(D, PPL, 65, nc.gpsimd)
                    if c >= 1:
                        r_chunk(PPL, RD, c, ENG[R_PAT[c % len(R_PAT)]])

                def finale():
                    # masked seam row 0 (reads D row 64 = chunk 15) ->
                    # P seam row 0 -> R(0); pass-A runs chunk 0 last
                    for pair in range(2):
                        q0, q1 = 2 * pair, 2 * pair + 1
                        nc.sync.dma_start(dg[32 * q1:32 * q1 + 32, 0, :],
                                          dg[32 * q0:32 * q0 + 32, 64, :])
                    p_seam(D, PPL, 0, nc.gpsimd)
                    r_chunk(PPL, RD, 0, ENG[R_PAT[0]])
                return step, finale

            def pass_a(S, D, RS, with_tail):
                """Taps/relu/W2/evict S->D, chunk order [1..15, 0].  If
                with_tail, interleave the mask phase of the PREVIOUS
                transition (which masks S and builds RS) two chunks ahead
                of the pass-A chunk that consumes it."""
                sg = S[:].rearrange("p (r c) -> p r c", c=COLS)
                sgf = S[:].bitcast(f32).rearrange("p (r c) -> p r c", c=COLS)
                dg = D[:].rearrange("p (r c) -> p r c", c=COLS)
                rsg = RS[:].rearrange("p (r c) -> p r c", c=COLS)
                tail = make_tail(S, RS) if with_tail else None

                # PE warm-keeper: dep-free matmuls bridging the prep-phase
                # PE idle gap so HAM doesn't re-throttle the clock
                if WARM:
                    jp = pp.tile([128, N], f32, tag="ps", name="jp")
                    for _ in range(WARM):
                        nc.tensor.matmul(jp[0:128, 0:384], ONESB[0:32, 0:128],
                                         WTSR[0:32, :],
                                         start=True, stop=True)
                if tail:
                    for c in range(min(AHEAD, NCH)):
                        tail[0](c)

                relu_k = 0
                pend = None          # (i, H tiles) awaiting W2+evict

                def w2_evict(p):
                    i, Hs = p
                    r0 = 1 + CR * i
                    cp = pp.tile([128, N], f32, tag="ps", name="cp")
                    for q in range(4):
                        nc.tensor.matmul(
                            cp[32 * q:32 * q + 32, :],
                            WTS2[0:96, 32 * q:32 * q + 32],
                            Hs[q][0:96, :],
                            start=True, stop=True,
                            tile_position=(0, 32 * q))
                    ENG[EVICT_PAT[i % len(EVICT_PAT)]].scalar_tensor_tensor(
                        dg[:, r0:r0 + CR, 1:1 + W],
                        cp[:, :], 0.0,
                        sgf[:, r0:r0 + CR, 1:1 + W],
                        op0=AT.add, op1=AT.add)

                for idx, i in enumerate(list(range(1, NCH)) + [0]):
                    r0 = 1 + CR * i
                    hps = [pp.tile([128, N], f32, tag="ps", name="hp")
                           for q in range(4)]
                    # 5 taps, t-major, weight-stationary per (t,q)
                    for t in range(5):
                        for q in range(4):
                            if t == 0:
                                lhsT = WTSC[32 * q:32 * q + 32, 0:96]
                                rhs = sg[32 * q:32 * q + 32,
                                         r0:r0 + CR, 1:1 + W]
                            else:
                                ty, tx = (t - 1) // 2, (t - 1) % 2
                                lhsT = WTSR[32 * q:32 * q + 32,
                                            (t - 1) * 96:(t - 1) * 96 + 96]
                                rhs = rsg[32 * q:32 * q + 32,
                                          r0 - 1 + ty:r0 - 1 + ty + CR,
                                          tx:tx + W]
                            nc.tensor.matmul(
                                hps[q][0:96, :], lhsT, rhs,
                                start=(t == 0), stop=(t == 4),
                                tile_position=(32 * q, 0))
                    # W2+evict of the previous chunk (its relus are done by
                    # now, so the PE never waits on ACT/DVE)
                    if pend is not None:
                        w2_evict(pend)
                    Hs = []
                    for q in range(4):
                        Ht = hbp.tile([128, N], bf16, tag="H", name="Ht")
                        do_relu(relu_k, Ht[0:96, :], hps[q][0:96, :])
                        relu_k += 1
                        Hs.append(Ht)
                    pend = (i, Hs)
                    if tail:
                        nxt = idx + AHEAD
                        if nxt < NCH:
                            tail[0](nxt)
                        elif nxt == NCH:
                            tail[1]()
                    if idx == 6 and with_tail:
                        b2path()
                    if idx == NCH - 1:
                        # wave-1 spread: hb rows 1..14 (needs evicts 1..15,
                        # which are all done; chunk 0 evicts after this)
                        engs = [nc.sync, nc.scalar]
                        k = 0
                        for pair in range(2):
                            for hh in range(2):
                                q = 2 * pair + hh
                                for g in range(2):
                                    j = 2 * pair + g
                                    p = 32 * q + 16 * g
                                    srcb = D[p:p + 1].bitcast(f32)
                                    src = bass.AP(
                                        srcb.tensor,
                                        srcb.offset + 4 * COLS,
                                        [list(srcb.ap[0]), [4 * COLS, 14],
                                         [1, 6 * COLS]])
                                    pb = 32 * j + 16 * hh + 1
                                    engs[k % 2].dma_start(
                                        B[pb:pb + 14, :]
                                        .rearrange("p (r c) -> p r c",
                                                   c=COLS), src)
                                    k += 1
                w2_evict(pend)

            def b2path():
                """Next-pre maxpool of the previous transition (reads old B
                and MB, writes PREB consumed by the next prep's MB op)."""
                nc.vector.tensor_tensor(
                    b2g[:, 1:5, 1:1 + W], bg[:, 1:5, 1:1 + W],
                    MB[:].rearrange("p (r c) -> p r c", c=W), op=AT.mult)
                for j in range(NIMG):
                    nc.sync.dma_start(b2g[32 * j:32 * j + 31, 5, :],
                                      b2g[32 * j + 1:32 * j + 32, 1, :])
                    nc.sync.dma_start(b2g[32 * j + 1:32 * j + 32, 0, :],
                                      b2g[32 * j:32 * j + 31, 4, :])
                nc.vector.tensor_tensor(VM[:, :], B2[:, 0:4 * COLS],
                                        B2[:, COLS:5 * COLS], op=AT.max)
                nc.vector.tensor_tensor(VM[:, :], VM[:, :],
                                        B2[:, 2 * COLS:6 * COLS], op=AT.max)
                nc.vector.tensor_tensor(hmg, vg[:, :, 0:W],
                                        vg[:, :, 1:1 + W], op=AT.max)
                nc.vector.tensor_tensor(hmg, hmg, vg[:, :, 2:2 + W],
                                        op=AT.max)
                nc.vector.tensor_single_scalar(PREB[:, :], HM[:, :],
                                               LIVE_THR, op=AT.is_gt)

            def prep(D, first=False):
                """Mask prep for D: unmasked seams, wave-2 ch0 spread,
                maxpool, MB = post&pre."""
                dg = D[:].rearrange("p (r c) -> p r c", c=COLS)
                # unmasked seams on D (for spread halos)
                for pair in range(2):
                    q0, q1 = 2 * pair, 2 * pair + 1
                    nc.sync.dma_start(dg[32 * q1:32 * q1 + 32, 0, :],
                                      dg[32 * q0:32 * q0 + 32, 64, :])
                    nc.sync.dma_start(dg[32 * q0:32 * q0 + 32, 65, :],
                                      dg[32 * q1:32 * q1 + 32, 1, :])
                # wave-2 spread: hb rows {0, 15} (needs seams + last evict)
                engs = [nc.sync, nc.scalar]
                k = 0
                for pair in range(2):
                    for hh in range(2):
                        q = 2 * pair + hh
                        for g in range(2):
                            j = 2 * pair + g
                            p = 32 * q + 16 * g
                            srcb = D[p:p + 1].bitcast(f32)
                            src = bass.AP(
                                srcb.tensor, srcb.offset,
                                [list(srcb.ap[0]), [60 * COLS, 2],
                                 [1, 6 * COLS]])
                            pb = 32 * j + 16 * hh
                            db = B[pb:pb + 1]
                            dst = bass.AP(
                                db.tensor, db.offset,
                                [[db.ap[0][0] * 15, 2], [1, 6 * COLS]])
                            engs[k % 2].dma_start(dst, src)
                            k += 1
                # maxpool + threshold, fused with pre -> MB
                nc.vector.tensor_tensor(VM[:, :], B[:, 0:4 * COLS],
                                        B[:, COLS:5 * COLS], op=AT.max)
                nc.vector.tensor_tensor(VM[:, :], VM[:, :],
                                        B[:, 2 * COLS:6 * COLS], op=AT.max)
                nc.vector.tensor_tensor(hmg, vg[:, :, 0:W], vg[:, :, 1:1 + W],
                                        op=AT.max)
                nc.vector.tensor_tensor(hmg, hmg, vg[:, :, 2:2 + W], op=AT.max)
                nc.vector.scalar_tensor_tensor(MB[:, :], HM[:, :], LIVE_THR,
                                               PREB[:, :],
                                               op0=AT.is_gt, op1=AT.mult)

            def plain_tail(S, RS):
                stepf, finalef = make_tail(S, RS)
                for c in range(NCH):
                    stepf(c)
                finalef()

            # prologue: P/R planes of the initial state + transition 1
            pr_planes(CELLA, PPL, RPLA)
            assert NITER >= 4 and NITER % 2 == 0
            nbody = (NITER - 2) // 2
            pass_a(CELLA, CELLB, RPLA, with_tail=False)
            prep(CELLB)
            if os.environ.get("CANET_LOOP", "fori") == "fori" and nbody > 1:
                with tc.For_i(0, nbody, 1, hint_engines=(
                        mybir.EngineType.PE, mybir.EngineType.DVE,
                        mybir.EngineType.Activation, mybir.EngineType.SP,
                        mybir.EngineType.Pool)):
                    pass_a(CELLB, CELLA, RPLB, with_tail=True)
                    prep(CELLA)
                    pass_a(CELLA, CELLB, RPLA, with_tail=True)
                    prep(CELLB)
            else:
                for _ in range(nbody):
                    pass_a(CELLB, CELLA, RPLB, with_tail=True)
                    prep(CELLA)
                    pass_a(CELLA, CELLB, RPLA, with_tail=True)
                    prep(CELLB)
            # epilogue: transition 30 + final mask
            pass_a(CELLB, CELLA, RPLB, with_tail=True)
            prep(CELLA)
            plain_tail(CELLA, RPLA)
            FINAL = CELLA

            fgf = FINAL[:].bitcast(f32).rearrange("p (r c) -> p r c", c=COLS)
            for pair in range(2):
                for hh in range(2):
                    q = 2 * pair + hh
                    for g in range(2):
                        j = 2 * pair + g
                        nc.sync.dma_start(
                            out_ext[j, :, 64 * hh:64 * hh + 64, :],
                            fgf[32 * q + 16 * g + 1:32 * q + 16 * g + 11,
                                1:65, 1:1 + W])

    nc.finalize()
    if os.environ.get("CANET_SPLIT", "1") == "1":
        _split_waits(nc)
    return nc


def _get_built():
    global _BUILT
    if _BUILT is None:
        _BUILT = _build()
    return _BUILT


def _host_inputs(inp, w1, w2):
    wtsc, wtsr, wts2, onesb = _host_weights(w1, w2)
    in_maps = []
    for core in range(NCORES):
        inp4 = inp[NIMG * core:NIMG * core + NIMG]
        cell0_dev, cell = _host_cell0(inp4)
        preb0 = _host_preb0(cell)
        in_maps.append({"cell0": cell0_dev, "wtsc": wtsc, "wtsr": wtsr,
                        "wts2": wts2, "onesb": onesb, "preb0": preb0})
    return in_maps


def kernel(inp, w1, w2):
    inp = np.asarray(inp, np.float32)
    w1 = np.asarray(w1, np.float32)
    w2 = np.asarray(w2, np.float32)
    from concourse.bass_utils import run_bass_kernel_spmd

    nc = _get_built()
    in_maps = _host_inputs(inp, w1, w2)
    res = run_bass_kernel_spmd(nc, in_maps, list(range(NCORES))).results
    out = np.concatenate([res[c]["out"] for c in range(NCORES)], axis=0)
    return np.ascontiguousarray(out.astype(np.float32))
